# revision 48
# baseline (speedup 1.0000x reference)
"""Trainium2 Bass kernel for CustomAttention (qkv -> per-head LN on q,k -> SDPA -> proj).

Sharding: 8 cores = 2 batches x 4 head-groups (3 heads each).
Per core: qkv projection for its heads from x[b], per-head LN applied as a
per-token affine (stats batched across all token blocks), full attention per
head (scores computed transposed; softmax denominator folded into the PV
matmul as a ones column on V), then a partial output projection over its 192
channels. Host sums the 4 partials per batch and adds proj_b.

exp is split between the ACT engine (exact, table-based) and the DVE
(one-instruction bit-trick exp: round(A*x+B) as int16 == the bf16 pattern of
2^(A'x+B')), per query-column so each softmax row uses one method
consistently.
"""

import os
import sys
from functools import lru_cache

import numpy as np

for _p in ("/opt/trn_rl_repo", os.path.expanduser("~/.axon_site/_ro/trn_rl_repo")):
    if os.path.isdir(_p) and _p not in sys.path:
        sys.path.insert(0, _p)

import concourse.bass as bass
import concourse.mybir as mybir
from concourse import bacc
import concourse.tile as tile
from concourse.masks import make_identity

F32 = mybir.dt.float32
BF16 = mybir.dt.bfloat16
I16 = mybir.dt.int16
ALU = mybir.AluOpType
ACTF = mybir.ActivationFunctionType
AX = mybir.AxisListType

H = 3          # heads per core
D = 64         # head dim
C = 768        # model dim
J = 3 * H * D + 6  # qkv cols per core = 576 qkv + 6 q/k row-sum cols (s1)
EPS = 1e-5
SCALE = D ** -0.5

# Schraudolph exp: bits(bf16) = rint(A16*x + B16) (HW converts with rint)
A16 = 128.0 / np.log(2.0)
B16 = 127.0 * 128.0 - 2.0

# columns (of each 512-wide i-block) computed on DVE with the bit-trick exp;
# the rest go to the ACT engine with exact exp.
FEXP = 240

# Phase-C software pipeline skew: PV matmuls run this many exp-tiles behind
# the score matmuls so PE never stalls on the exp producers.
SKEW = 11


def build_nc(N=4096, plain_ln=True, fexp=FEXP):
    """One-core program; all 8 cores run it SPMD with different input data."""
    NB = N // 128          # n-blocks / j-chunks
    IB = N // 512          # i-blocks
    NHALF = NB // 2        # j-chunk pairs for row-tiled score matmuls

    nc = bacc.Bacc("TRN2", target_bir_lowering=False, debug=False)
    x_t = nc.declare_dram_parameter("x_t", [C, N], BF16, isOutput=False)
    wqkv_t = nc.declare_dram_parameter("wqkv_t", [C, J], BF16, isOutput=False)
    projw_t = nc.declare_dram_parameter("projw_t", [H * D, C], BF16, isOutput=False)
    gb = nc.declare_dram_parameter("gb", [4, D], F32, isOutput=False)
    out_p = nc.declare_dram_parameter("out_p", [N, C], F32, isOutput=True)

    with tile.TileContext(nc) as tc:
        with (
            tc.tile_pool(name="persist", bufs=1) as persist,
            tc.tile_pool(name="weights", bufs=1) as weights,
        ):
            # ---- persistent SBUF tensors ----
            # qT duplicated across both partition halves: rows 0:64 == 64:128
            qT = persist.tile([128, H, N], BF16, tag="qT")
            # kT stacked: rows 0:64 = j in [0,N/2), rows 64:128 = j in [N/2,N)
            kT = persist.tile([128, H, N // 2], BF16, tag="kT")
            # V augmented with a ones column (index 64) per j-chunk
            vA = persist.tile([128, H, NB, 65], BF16, tag="vA")
            # attention output (bf16): ao1 rows 0:64 h0, 64:128 h1; ao2 h2
            ao1 = persist.tile([128, N], BF16, tag="ao1")
            ao2 = persist.tile([64, N], BF16, tag="ao2")
            # qkv staging (bf16) for q,k; stats and per-token affine coeffs
            scr = persist.tile([128, NB, 2 * 192], BF16, tag="scr")
            stats = persist.tile([128, NB, 12], F32, tag="stats")
            aco = persist.tile([128, NB, 6], F32, tag="aco")
            bco = persist.tile([128, NB, 6], F32, tag="bco")

            ident = persist.tile([128, 128], F32, tag="ident")
            make_identity(nc, ident[:])
            identb = persist.tile([128, 128], BF16, tag="identb")
            nc.vector.tensor_copy(identb[:], ident[:])
            nc.vector.memset(vA[:, :, :, 64:65], 1.0)

            wq = weights.tile([128, 6, J], BF16, tag="wqkv")
            nc.sync.dma_start(
                wq[:], wqkv_t.rearrange("(ck p) j -> p ck j", p=128)
            )
            pw128 = weights.tile([128, C], BF16, tag="pw128")
            nc.sync.dma_start(pw128[:], projw_t[0:128, :])
            pw64 = weights.tile([64, C], BF16, tag="pw64")
            nc.sync.dma_start(pw64[:], projw_t[128:192, :])
            # gamma/beta broadcast across partitions: [gq, bq, gk, bk]
            gbt = weights.tile([128, 4, D], F32, tag="gb")
            nc.sync.dma_start(gbt[:], gb[None, :, :].to_broadcast([128, 4, D]))

            # ====== Phase B: qkv + LN + transpose, rolling groups of 4 ======
            G4 = 8
            with (
                tc.tile_pool(name="pB", bufs=6) as pB,
                tc.tile_pool(name="pB2", bufs=3) as pB2,
                tc.tile_pool(name="pB3", bufs=4) as pB3,
                tc.tile_pool(name="psB", bufs=3, space="PSUM") as psB,
                tc.tile_pool(name="psT", bufs=2, space="PSUM") as psT,
            ):
                epst = pB2.tile([128, 1], F32, tag="epst")
                nc.vector.memset(epst[:], 4096.0 * EPS)

                def b1(nb):
                    xt = pB.tile([128, 6, 128], BF16, tag="xt")
                    nc.sync.dma_start(
                        xt[:],
                        x_t.rearrange("(ck p) n -> p ck n", p=128)[
                            :, :, nb * 128 : (nb + 1) * 128
                        ],
                    )
                    # half0 = q(192)+k(192); half1 = v(192) + s1(6)
                    ps0 = psB.tile([128, 384], F32, tag="qkvps0")
                    ps1 = psB.tile([128, 198], F32, tag="qkvps1")
                    for (ps, lo, hi) in ((ps0, 0, 384), (ps1, 384, 582)):
                        for ck in range(6):
                            nc.tensor.matmul(
                                ps[:],
                                xt[:, ck, :],
                                wq[:, ck, lo:hi],
                                start=(ck == 0),
                                stop=(ck == 5),
                            )
                    # qk -> bf16 scratch, split across DVE and ACT
                    nc.vector.tensor_copy(scr[:, nb, 0:192], ps0[:, 0:192])
                    nc.scalar.copy(scr[:, nb, 192:384], ps0[:, 192:384])
                    # v -> vA, one strided copy for all 3 heads
                    nc.scalar.copy(
                        vA[:, :, nb, 0:64],
                        ps1[:, 0:192].rearrange("p (h d) -> p h d", d=64),
                    )
                    # s1 came out of the matmul (row-sum weight cols)
                    nc.scalar.copy(stats[:, nb, 0:6], ps1[:, 192:198])
                    # s2: square (gpsimd) then free-axis reduce (DVE)
                    src3 = scr[:, nb, :].rearrange("p (g d) -> p g d", d=64)
                    sq = pB.tile([128, 6, 64], BF16, tag="sq")
                    nc.gpsimd.tensor_mul(sq[:], src3, src3)
                    nc.vector.tensor_reduce(
                        stats[:, nb, 6:12], sq[:], AX.X, ALU.add
                    )

                def b2(g):
                    gsl = slice(g * G4, (g + 1) * G4)
                    s1 = stats[:, gsl, 0:6]
                    s2 = stats[:, gsl, 6:12]
                    musq = pB2.tile([128, G4, 6], F32, tag="musq")
                    nc.vector.tensor_mul(musq[:], s1, s1)
                    v4k = pB2.tile([128, G4, 6], F32, tag="v4k")
                    # 4096*var = 64*s2 - s1^2
                    nc.vector.scalar_tensor_tensor(
                        v4k[:], s2, 64.0, musq[:], ALU.mult, ALU.subtract
                    )
                    std = pB2.tile([128, G4, 6], F32, tag="std")
                    nc.scalar.activation(std[:], v4k[:], ACTF.Sqrt, bias=epst[:])
                    r0 = pB2.tile([128, G4, 6], F32, tag="r0")
                    nc.vector.reciprocal(r0[:], std[:])  # = rstd/64
                    # a = rstd*scale (q: scale/8, k: 1) ; b = -mu*a
                    nc.vector.tensor_scalar_mul(
                        aco[:, gsl, 0:3], r0[:, :, 0:3], 64.0 * SCALE
                    )
                    nc.vector.tensor_scalar_mul(aco[:, gsl, 3:6], r0[:, :, 3:6], 64.0)
                    nc.vector.scalar_tensor_tensor(
                        bco[:, gsl], s1, -1.0 / 64.0, aco[:, gsl], ALU.mult, ALU.mult
                    )

                def b3(nb):
                    ap = pB3.tile([128, 2, H, D], BF16, tag="apl")
                    for t in range(2):
                        for h in range(H):
                            cix = 3 * t + h
                            eng = nc.vector if h < 2 else nc.gpsimd
                            eng.tensor_scalar(
                                ap[:, t, h, :],
                                scr[:, nb, 192 * t + 64 * h : 192 * t + 64 * h + 64],
                                aco[:, nb, cix : cix + 1],
                                bco[:, nb, cix : cix + 1],
                                ALU.mult,
                                ALU.add,
                            )
                    if not plain_ln:
                        for t in range(2):
                            g3 = gbt[:, 2 * t, :][:, None, :].broadcast_to([128, H, D])
                            b3_ = gbt[:, 2 * t + 1, :][:, None, :].broadcast_to([128, H, D])
                            nc.vector.tensor_mul(ap[:, t], ap[:, t], g3)
                            nc.vector.tensor_add(ap[:, t], ap[:, t], b3_)
                    # transposes: q dup'd into both halves, k into its stack row
                    jh = nb // NHALF
                    cb = nb % NHALF
                    ptq = psT.tile([128, 2 * H, 128], BF16, tag="ptq")
                    for h in range(H):
                        nc.tensor.transpose(ptq[0:64, h, :], ap[:, 0, h, :], identb[:])
                        nc.tensor.transpose(
                            ptq[64 * jh : 64 * jh + 64, H + h, :],
                            ap[:, 1, h, :],
                            identb[:],
                        )
                    blk = slice(nb * 128, (nb + 1) * 128)
                    nc.scalar.copy(qT[0:64, :, blk], ptq[0:64, 0:H, :])
                    # duplicate q rows into the upper partition half via DMA
                    nc.sync.dma_start(qT[64:128, :, blk], qT[0:64, :, blk])
                    nc.vector.tensor_copy(
                        kT[64 * jh : 64 * jh + 64, :, cb * 128 : (cb + 1) * 128],
                        ptq[64 * jh : 64 * jh + 64, H : 2 * H, :],
                    )

                # rolling schedule: b1(g) ; b2(g-1) ; b3(g-1) interleaved with b1
                NG = NB // G4
                for g in range(NG + 1):
                    if g < NG:
                        for k in range(G4):
                            b1(g * G4 + k)
                            if g > 0:
                                b3((g - 1) * G4 + k)
                    else:
                        for k in range(G4):
                            b3((g - 1) * G4 + k)
                    if g < NG:
                        b2(g)

            # ================= Phase C: attention =================
            with (
                tc.tile_pool(name="pt", bufs=SKEW + 3) as ptp,
                tc.tile_pool(name="pCs", bufs=4) as pCs,
                tc.tile_pool(name="pD", bufs=3) as pD,
                tc.tile_pool(name="psS", bufs=2, space="PSUM") as psS,
                tc.tile_pool(name="psO", bufs=2, space="PSUM") as psO,
                tc.tile_pool(name="psD", bufs=2, space="PSUM") as psD,
            ):
                def emit_proj(nb):
                    blk = slice(nb * 128, (nb + 1) * 128)
                    stage = pD.tile([128, C], F32, tag="stage")
                    for oc, osz in ((0, 512), (512, 256)):
                        ps = psD.tile([128, 512], F32, tag="pd")
                        nc.tensor.matmul(
                            ps[:, 0:osz],
                            ao1[:, blk],
                            pw128[:, oc : oc + osz],
                            start=True,
                            stop=False,
                        )
                        nc.tensor.matmul(
                            ps[:, 0:osz],
                            ao2[0:64, blk],
                            pw64[0:64, oc : oc + osz],
                            start=False,
                            stop=True,
                        )
                        if osz == 512:
                            nc.scalar.copy(stage[:, oc : oc + osz], ps[:, 0:osz])
                        else:
                            nc.vector.tensor_copy(stage[:, oc : oc + osz], ps[:, 0:osz])
                    nc.sync.dma_start(out_p[blk, :], stage[:])

                queue = []
                deferred = []

                def finish(ctx):
                    # cheap stage now; the big normalize muls are deferred in
                    # halves so they interleave with later exp ops on DVE
                    # instead of stalling the exp pipeline in one burst
                    pso, h, isl = ctx["pso"], ctx["h"], ctx["isl"]
                    rden_f = pCs.tile([1, 512], F32, tag="rden_f")
                    nc.scalar.copy(rden_f[:], pso[64:65, :])
                    rden = pCs.tile([1, 512], F32, tag="rden")
                    nc.vector.reciprocal_approx_fast(rden[:], rden_f[:])
                    rb = pCs.tile([64, 512], F32, tag="rb")
                    nc.gpsimd.partition_broadcast(rb[:], rden[:])
                    i0 = isl.start
                    if h == 1:
                        stg = pCs.tile([64, 512], BF16, tag="stg")

                        def mul1(c):
                            nc.vector.tensor_mul(
                                stg[:, c : c + 256],
                                pso[0:64, c : c + 256],
                                rb[:, c : c + 256],
                            )

                        def fire():
                            nc.sync.dma_start(ao1[64:128, isl], stg[:])

                        deferred.extend([lambda: mul1(0), lambda: mul1(256), fire])
                    else:
                        dst = ao1 if h == 0 else ao2

                        def mul2(c, dst=dst):
                            nc.vector.tensor_mul(
                                dst[0:64, i0 + c : i0 + c + 256],
                                pso[0:64, c : c + 256],
                                rb[:, c : c + 256],
                            )

                        deferred.extend([lambda: mul2(0), lambda: mul2(256)])

                def emit_pv():
                    pt_half, jc, ctx = queue.pop(0)
                    nc.tensor.matmul(
                        ctx["pso"][:],
                        vA[:, ctx["h"], jc, :],
                        pt_half,
                        start=(ctx["n"] == 0),
                        stop=(ctx["n"] == NB - 1),
                    )
                    ctx["n"] += 1
                    if ctx["n"] == NB:
                        finish(ctx)
                    elif deferred:
                        deferred.pop(0)()

                for ib in range(IB):
                    for h in range(H):
                        isl = slice(ib * 512, (ib + 1) * 512)
                        pso = psO.tile([65, 512], F32, tag="pso")
                        ctx = {"pso": pso, "h": h, "isl": isl, "n": 0}

                        for jp in range(NHALF):
                            # previous i-block's proj, spread through this
                            # stream; must come after finish() of the previous
                            # block's last head (drained by jp ~= SKEW here)
                            # plus the deferred normalize chunks
                            if ib > 0 and h == 0 and jp >= 13:
                                emit_proj((ib - 1) * 4 + (jp - 13))
                            if ib > 0 and h == 1 and jp == 0:
                                emit_proj((ib - 1) * 4 + 3)
                            jsl = slice(jp * 128, (jp + 1) * 128)
                            ps = psS.tile([128, 1024], F32, tag="st")
                            for half in range(2):
                                psl = slice(64 * half, 64 * half + 64)
                                nc.tensor.matmul(
                                    ps[:, 512 * half : 512 * half + 512],
                                    kT[psl, h, jsl],
                                    qT[psl, h, isl],
                                    start=True,
                                    stop=True,
                                    tile_position=(64 * half, 0),
                                )
                            pt = ptp.tile([128, 2, 512], BF16, tag="pt")
                            ps3 = ps[:].rearrange("p (g i) -> p g i", g=2)
                            if fexp > 0:
                                nc.vector.tensor_scalar(
                                    pt[:, :, 0:fexp].bitcast(I16),
                                    ps3[:, :, 0:fexp],
                                    A16,
                                    B16,
                                    ALU.mult,
                                    ALU.add,
                                )
                            nc.scalar.activation(
                                pt[:, :, fexp:512], ps3[:, :, fexp:512], ACTF.Exp
                            )
                            ptf = pt[:].rearrange("p g i -> p (g i)")
                            queue.append((ptf[:, 0:512], jp, ctx))
                            queue.append((ptf[:, 512:1024], jp + NHALF, ctx))
                            while len(queue) > 2 * SKEW:
                                emit_pv()
                while queue:
                    emit_pv()
                while deferred:
                    deferred.pop(0)()
                # last i-block's proj
                for nb in range((IB - 1) * 4, IB * 4):
                    emit_proj(nb)

    nc.compile()
    return nc


@lru_cache(maxsize=2)
def _built(N, plain_ln=True):
    return build_nc(N, plain_ln=plain_ln)


def _prep_inputs(x, qkv_w, q_gamma, q_beta, k_gamma, k_beta, proj_w):
    x = np.asarray(x, np.float32)
    qkv_w = np.asarray(qkv_w, np.float32)
    proj_w = np.asarray(proj_w, np.float32)
    B = x.shape[0]
    import ml_dtypes
    xts = [np.ascontiguousarray(x[b].T).astype(ml_dtypes.bfloat16) for b in range(B)]
    gbs = []
    wqs = []
    pws = []
    for g in range(4):
        r = slice(192 * g, 192 * (g + 1))
        wq_rows = np.concatenate(
            [qkv_w[r], qkv_w[768:1536][r], qkv_w[1536:2304][r]], axis=0
        )
        # 6 extra columns: per-(q/k, head) row sums of W -> s1 = sum_d qkv
        wbar = np.stack(
            [wq_rows[64 * g : 64 * (g + 1)].sum(axis=0) for g in range(6)]
        )
        wq_rows = np.concatenate([wq_rows, wbar], axis=0)
        wqs.append(np.ascontiguousarray(wq_rows.T).astype(ml_dtypes.bfloat16))
        pws.append(np.ascontiguousarray(proj_w[:, r].T).astype(ml_dtypes.bfloat16))
        gbs.append(
            np.stack(
                [
                    np.asarray(q_gamma, np.float32),
                    np.asarray(q_beta, np.float32),
                    np.asarray(k_gamma, np.float32),
                    np.asarray(k_beta, np.float32),
                ]
            )
        )
    in_maps = []
    for core in range(8):
        b, g = core // 4, core % 4
        in_maps.append(
            {"x_t": xts[b], "wqkv_t": wqs[g], "projw_t": pws[g], "gb": gbs[g]}
        )
    return in_maps


def _is_plain_ln(q_gamma, q_beta, k_gamma, k_beta):
    return (
        np.all(np.asarray(q_gamma) == 1.0)
        and np.all(np.asarray(k_gamma) == 1.0)
        and np.all(np.asarray(q_beta) == 0.0)
        and np.all(np.asarray(k_beta) == 0.0)
    )


def run_cores(in_maps, N, trace=False, plain_ln=True):
    from concourse.bass_utils import run_bass_kernel_spmd

    nc = _built(N, plain_ln)
    res = run_bass_kernel_spmd(nc, in_maps, list(range(8)), trace=trace)
    return res


def kernel(x, qkv_w, q_gamma, q_beta, k_gamma, k_beta, proj_w, proj_b):
    x = np.asarray(x, np.float32)
    N = x.shape[1]
    plain = _is_plain_ln(q_gamma, q_beta, k_gamma, k_beta)
    in_maps = _prep_inputs(x, qkv_w, q_gamma, q_beta, k_gamma, k_beta, proj_w)
    res = run_cores(in_maps, N, plain_ln=plain)
    parts = [np.asarray(r["out_p"], np.float32) for r in res.results]
    out0 = parts[0] + parts[1] + parts[2] + parts[3]
    out1 = parts[4] + parts[5] + parts[6] + parts[7]
    out = np.stack([out0, out1]) + np.asarray(proj_b, np.float32)
    return out.astype(np.float32)


# revision 49
# speedup vs baseline: 1.1981x; 1.1981x over previous
"""Trainium2 Bass kernel for CustomAttention (qkv -> per-head LN on q,k -> SDPA -> proj).

Sharding: 8 cores = 2 batches x 4 head-groups (3 heads each).
Per core: qkv projection for its heads from x[b], per-head LN applied as a
per-token affine (stats batched across all token blocks), full attention per
head (scores computed transposed; softmax denominator folded into the PV
matmul as a ones column on V), then a partial output projection over its 192
channels. Host sums the 4 partials per batch and adds proj_b.

exp is split between the ACT engine (exact, table-based) and the DVE
(one-instruction bit-trick exp: round(A*x+B) as int16 == the bf16 pattern of
2^(A'x+B')), per query-column so each softmax row uses one method
consistently.
"""

import os
import sys
from functools import lru_cache

import numpy as np

for _p in ("/opt/trn_rl_repo", os.path.expanduser("~/.axon_site/_ro/trn_rl_repo")):
    if os.path.isdir(_p) and _p not in sys.path:
        sys.path.insert(0, _p)

import concourse.bass as bass
import concourse.mybir as mybir
from concourse import bacc
import concourse.tile as tile
from concourse.masks import make_identity

F32 = mybir.dt.float32
BF16 = mybir.dt.bfloat16
I16 = mybir.dt.int16
ALU = mybir.AluOpType
ACTF = mybir.ActivationFunctionType
AX = mybir.AxisListType

H = 3          # heads per core
D = 64         # head dim
C = 768        # model dim
J = 3 * H * D + 6  # qkv cols per core = 576 qkv + 6 q/k row-sum cols (s1)
EPS = 1e-5
SCALE = D ** -0.5

# Schraudolph exp: bits(bf16) = rint(A16*x + B16) (HW converts with rint)
A16 = 128.0 / np.log(2.0)
B16 = 127.0 * 128.0 - 2.0

# columns (of each 512-wide i-block) computed on DVE with the bit-trick exp;
# the rest go to the ACT engine with exact exp.
FEXP = 240

# Phase-C software pipeline skew: PV matmuls run this many exp-tiles behind
# the score matmuls so PE never stalls on the exp producers.
SKEW = 11


def build_nc(N=4096, plain_ln=True, fexp=FEXP):
    """One-core program; all 8 cores run it SPMD with different input data."""
    NB = N // 128          # n-blocks / j-chunks
    IB = N // 512          # i-blocks
    NHALF = NB // 2        # j-chunk pairs for row-tiled score matmuls

    nc = bacc.Bacc("TRN2", target_bir_lowering=False, debug=False)
    x_t = nc.declare_dram_parameter("x_t", [C, N], BF16, isOutput=False)
    wqkv_t = nc.declare_dram_parameter("wqkv_t", [C, J], BF16, isOutput=False)
    projw_t = nc.declare_dram_parameter("projw_t", [H * D, C], BF16, isOutput=False)
    gb = nc.declare_dram_parameter("gb", [4, D], F32, isOutput=False)
    out_p = nc.declare_dram_parameter("out_p", [N, C], F32, isOutput=True)

    with tile.TileContext(nc) as tc:
        with (
            tc.tile_pool(name="persist", bufs=1) as persist,
            tc.tile_pool(name="weights", bufs=1) as weights,
        ):
            # ---- persistent SBUF tensors ----
            # qT duplicated across both partition halves: rows 0:64 == 64:128
            qT = persist.tile([128, H, N], BF16, tag="qT")
            # kT stacked: rows 0:64 = j in [0,N/2), rows 64:128 = j in [N/2,N)
            kT = persist.tile([128, H, N // 2], BF16, tag="kT")
            # V augmented with a ones column (index 64) per j-chunk
            vA = persist.tile([128, H, NB, 65], BF16, tag="vA")
            # attention output (bf16): ao1 rows 0:64 h0, 64:128 h1; ao2 h2
            ao1 = persist.tile([128, N], BF16, tag="ao1")
            ao2 = persist.tile([64, N], BF16, tag="ao2")
            # qkv staging (bf16) for q,k; stats and per-token affine coeffs
            scr = persist.tile([128, NB, 2 * 192], BF16, tag="scr")
            stats = persist.tile([128, NB, 12], F32, tag="stats")
            aco = persist.tile([128, NB, 6], F32, tag="aco")
            bco = persist.tile([128, NB, 6], F32, tag="bco")

            ident = persist.tile([128, 128], F32, tag="ident")
            make_identity(nc, ident[:])
            identb = persist.tile([128, 128], BF16, tag="identb")
            nc.vector.tensor_copy(identb[:], ident[:])
            nc.vector.memset(vA[:, :, :, 64:65], 1.0)

            wq = weights.tile([128, 6, J], BF16, tag="wqkv")
            nc.sync.dma_start(
                wq[:], wqkv_t.rearrange("(ck p) j -> p ck j", p=128)
            )
            pw128 = weights.tile([128, C], BF16, tag="pw128")
            nc.sync.dma_start(pw128[:], projw_t[0:128, :])
            pw64 = weights.tile([64, C], BF16, tag="pw64")
            nc.sync.dma_start(pw64[:], projw_t[128:192, :])
            # gamma/beta broadcast across partitions: [gq, bq, gk, bk]
            gbt = weights.tile([128, 4, D], F32, tag="gb")
            nc.sync.dma_start(gbt[:], gb[None, :, :].to_broadcast([128, 4, D]))

            # ====== Phase B: qkv + LN + transpose, rolling groups of 4 ======
            G4 = 8
            with (
                tc.tile_pool(name="pB", bufs=4) as pB,
                tc.tile_pool(name="pB2", bufs=3) as pB2,
                tc.tile_pool(name="pB3", bufs=4) as pB3,
                tc.tile_pool(name="psB", bufs=3, space="PSUM") as psB,
                tc.tile_pool(name="psT", bufs=2, space="PSUM") as psT,
            ):
                epst = pB2.tile([128, 1], F32, tag="epst")
                nc.vector.memset(epst[:], 4096.0 * EPS)

                def b1(nb):
                    xt = pB.tile([128, 6, 128], BF16, tag="xt")
                    nc.sync.dma_start(
                        xt[:],
                        x_t.rearrange("(ck p) n -> p ck n", p=128)[
                            :, :, nb * 128 : (nb + 1) * 128
                        ],
                    )
                    # half0 = q(192)+k(192); half1 = v(192) + s1(6)
                    ps0 = psB.tile([128, 384], F32, tag="qkvps0")
                    ps1 = psB.tile([128, 198], F32, tag="qkvps1")
                    for (ps, lo, hi) in ((ps0, 0, 384), (ps1, 384, 582)):
                        for ck in range(6):
                            nc.tensor.matmul(
                                ps[:],
                                xt[:, ck, :],
                                wq[:, ck, lo:hi],
                                start=(ck == 0),
                                stop=(ck == 5),
                            )
                    # qk -> bf16 scratch, split across DVE and ACT
                    nc.vector.tensor_copy(scr[:, nb, 0:192], ps0[:, 0:192])
                    nc.scalar.copy(scr[:, nb, 192:384], ps0[:, 192:384])
                    # v -> vA, one strided copy for all 3 heads
                    nc.scalar.copy(
                        vA[:, :, nb, 0:64],
                        ps1[:, 0:192].rearrange("p (h d) -> p h d", d=64),
                    )
                    # s1 came out of the matmul (row-sum weight cols)
                    nc.scalar.copy(stats[:, nb, 0:6], ps1[:, 192:198])
                    # s2: square (gpsimd) then free-axis reduce (DVE)
                    src3 = scr[:, nb, :].rearrange("p (g d) -> p g d", d=64)
                    sq = pB.tile([128, 6, 64], BF16, tag="sq")
                    nc.gpsimd.tensor_mul(sq[:], src3, src3)
                    nc.vector.tensor_reduce(
                        stats[:, nb, 6:12], sq[:], AX.X, ALU.add
                    )

                def b2(g):
                    gsl = slice(g * G4, (g + 1) * G4)
                    s1 = stats[:, gsl, 0:6]
                    s2 = stats[:, gsl, 6:12]
                    musq = pB2.tile([128, G4, 6], F32, tag="musq")
                    nc.vector.tensor_mul(musq[:], s1, s1)
                    v4k = pB2.tile([128, G4, 6], F32, tag="v4k")
                    # 4096*var = 64*s2 - s1^2
                    nc.vector.scalar_tensor_tensor(
                        v4k[:], s2, 64.0, musq[:], ALU.mult, ALU.subtract
                    )
                    std = pB2.tile([128, G4, 6], F32, tag="std")
                    nc.scalar.activation(std[:], v4k[:], ACTF.Sqrt, bias=epst[:])
                    r0 = pB2.tile([128, G4, 6], F32, tag="r0")
                    nc.vector.reciprocal(r0[:], std[:])  # = rstd/64
                    # a = rstd*scale (q: scale/8, k: 1) ; b = -mu*a
                    nc.vector.tensor_scalar_mul(
                        aco[:, gsl, 0:3], r0[:, :, 0:3], 64.0 * SCALE
                    )
                    nc.vector.tensor_scalar_mul(aco[:, gsl, 3:6], r0[:, :, 3:6], 64.0)
                    nc.vector.scalar_tensor_tensor(
                        bco[:, gsl], s1, -1.0 / 64.0, aco[:, gsl], ALU.mult, ALU.mult
                    )

                def b3(nb):
                    ap = pB3.tile([128, 2, H, D], BF16, tag="apl")
                    for t in range(2):
                        for h in range(H):
                            cix = 3 * t + h
                            eng = nc.vector if h < 2 else nc.gpsimd
                            eng.tensor_scalar(
                                ap[:, t, h, :],
                                scr[:, nb, 192 * t + 64 * h : 192 * t + 64 * h + 64],
                                aco[:, nb, cix : cix + 1],
                                bco[:, nb, cix : cix + 1],
                                ALU.mult,
                                ALU.add,
                            )
                    if not plain_ln:
                        for t in range(2):
                            g3 = gbt[:, 2 * t, :][:, None, :].broadcast_to([128, H, D])
                            b3_ = gbt[:, 2 * t + 1, :][:, None, :].broadcast_to([128, H, D])
                            nc.vector.tensor_mul(ap[:, t], ap[:, t], g3)
                            nc.vector.tensor_add(ap[:, t], ap[:, t], b3_)
                    # transposes: q dup'd into both halves, k into its stack row
                    jh = nb // NHALF
                    cb = nb % NHALF
                    ptq = psT.tile([128, 2 * H, 128], BF16, tag="ptq")
                    for h in range(H):
                        nc.tensor.transpose(ptq[0:64, h, :], ap[:, 0, h, :], identb[:])
                        nc.tensor.transpose(
                            ptq[64 * jh : 64 * jh + 64, H + h, :],
                            ap[:, 1, h, :],
                            identb[:],
                        )
                    blk = slice(nb * 128, (nb + 1) * 128)
                    nc.scalar.copy(qT[0:64, :, blk], ptq[0:64, 0:H, :])
                    # duplicate q rows into the upper partition half via DMA
                    nc.sync.dma_start(qT[64:128, :, blk], qT[0:64, :, blk])
                    nc.vector.tensor_copy(
                        kT[64 * jh : 64 * jh + 64, :, cb * 128 : (cb + 1) * 128],
                        ptq[64 * jh : 64 * jh + 64, H : 2 * H, :],
                    )

                # rolling schedule: b1(g) ; b2(g-1) ; b3(g-1) interleaved with b1
                NG = NB // G4
                for g in range(NG + 1):
                    if g < NG:
                        for k in range(G4):
                            b1(g * G4 + k)
                            if g > 0:
                                b3((g - 1) * G4 + k)
                    else:
                        for k in range(G4):
                            b3((g - 1) * G4 + k)
                    if g < NG:
                        b2(g)

            # ================= Phase C: attention =================
            with (
                tc.tile_pool(name="pt", bufs=SKEW + 3) as ptp,
                tc.tile_pool(name="pCs", bufs=4) as pCs,
                tc.tile_pool(name="pD", bufs=3) as pD,
                tc.tile_pool(name="psS", bufs=2, space="PSUM") as psS,
                tc.tile_pool(name="psO", bufs=2, space="PSUM") as psO,
                tc.tile_pool(name="psD", bufs=2, space="PSUM") as psD,
            ):
                def emit_proj(nb):
                    blk = slice(nb * 128, (nb + 1) * 128)
                    stage = pD.tile([128, C], F32, tag="stage")
                    for oc, osz in ((0, 512), (512, 256)):
                        ps = psD.tile([128, 512], F32, tag="pd")
                        nc.tensor.matmul(
                            ps[:, 0:osz],
                            ao1[:, blk],
                            pw128[:, oc : oc + osz],
                            start=True,
                            stop=False,
                        )
                        nc.tensor.matmul(
                            ps[:, 0:osz],
                            ao2[0:64, blk],
                            pw64[0:64, oc : oc + osz],
                            start=False,
                            stop=True,
                        )
                        if osz == 512:
                            nc.scalar.copy(stage[:, oc : oc + osz], ps[:, 0:osz])
                        else:
                            nc.vector.tensor_copy(stage[:, oc : oc + osz], ps[:, 0:osz])
                    nc.sync.dma_start(out_p[blk, :], stage[:])

                queue = []
                deferred = []

                def finish(ctx):
                    # cheap stage now; the big normalize muls are deferred in
                    # halves so they interleave with later exp ops on DVE
                    # instead of stalling the exp pipeline in one burst
                    pso, h, isl = ctx["pso"], ctx["h"], ctx["isl"]
                    rden_f = pCs.tile([1, 512], F32, tag="rden_f")
                    nc.scalar.copy(rden_f[:], pso[64:65, :])
                    rden = pCs.tile([1, 512], F32, tag="rden")
                    nc.vector.reciprocal_approx_fast(rden[:], rden_f[:])
                    rb = pCs.tile([64, 512], F32, tag="rb")
                    nc.gpsimd.partition_broadcast(rb[:], rden[:])
                    i0 = isl.start
                    if h == 1:
                        stg = pCs.tile([64, 512], BF16, tag="stg")

                        def mul1(c):
                            nc.vector.tensor_mul(
                                stg[:, c : c + 256],
                                pso[0:64, c : c + 256],
                                rb[:, c : c + 256],
                            )

                        def fire():
                            nc.sync.dma_start(ao1[64:128, isl], stg[:])

                        deferred.extend([lambda: mul1(0), lambda: mul1(256), fire])
                    else:
                        dst = ao1 if h == 0 else ao2

                        def mul2(c, dst=dst):
                            nc.vector.tensor_mul(
                                dst[0:64, i0 + c : i0 + c + 256],
                                pso[0:64, c : c + 256],
                                rb[:, c : c + 256],
                            )

                        deferred.extend([lambda: mul2(0), lambda: mul2(256)])

                def emit_pv():
                    pt_half, jc, ctx = queue.pop(0)
                    nc.tensor.matmul(
                        ctx["pso"][:],
                        vA[:, ctx["h"], jc, :],
                        pt_half,
                        start=(ctx["n"] == 0),
                        stop=(ctx["n"] == NB - 1),
                    )
                    ctx["n"] += 1
                    if ctx["n"] == NB:
                        finish(ctx)
                    elif deferred:
                        deferred.pop(0)()

                for ib in range(IB):
                    for h in range(H):
                        isl = slice(ib * 512, (ib + 1) * 512)
                        pso = psO.tile([65, 512], F32, tag="pso")
                        ctx = {"pso": pso, "h": h, "isl": isl, "n": 0}

                        for jp in range(NHALF):
                            # previous i-block's proj, spread through this
                            # stream; must come after finish() of the previous
                            # block's last head (drained by jp ~= SKEW here)
                            # plus the deferred normalize chunks
                            if ib > 0 and h == 0 and jp >= 13:
                                emit_proj((ib - 1) * 4 + (jp - 13))
                            if ib > 0 and h == 1 and jp == 0:
                                emit_proj((ib - 1) * 4 + 3)
                            jsl = slice(jp * 128, (jp + 1) * 128)
                            ps = psS.tile([128, 1024], F32, tag="st")
                            for half in range(2):
                                psl = slice(64 * half, 64 * half + 64)
                                nc.tensor.matmul(
                                    ps[:, 512 * half : 512 * half + 512],
                                    kT[psl, h, jsl],
                                    qT[psl, h, isl],
                                    start=True,
                                    stop=True,
                                    tile_position=(64 * half, 0),
                                )
                            pt = ptp.tile([128, 2, 512], BF16, tag="pt")
                            ps3 = ps[:].rearrange("p (g i) -> p g i", g=2)
                            if fexp > 0:
                                nc.vector.tensor_scalar(
                                    pt[:, :, 0:fexp].bitcast(I16),
                                    ps3[:, :, 0:fexp],
                                    A16,
                                    B16,
                                    ALU.mult,
                                    ALU.add,
                                )
                            nc.scalar.activation(
                                pt[:, :, fexp:512], ps3[:, :, fexp:512], ACTF.Exp
                            )
                            ptf = pt[:].rearrange("p g i -> p (g i)")
                            queue.append((ptf[:, 0:512], jp, ctx))
                            queue.append((ptf[:, 512:1024], jp + NHALF, ctx))
                            while len(queue) > 2 * SKEW:
                                emit_pv()
                while queue:
                    emit_pv()
                while deferred:
                    deferred.pop(0)()
                # last i-block's proj
                for nb in range((IB - 1) * 4, IB * 4):
                    emit_proj(nb)

    nc.compile()
    return nc


@lru_cache(maxsize=2)
def _built(N, plain_ln=True):
    return build_nc(N, plain_ln=plain_ln)


def _prep_inputs(x, qkv_w, q_gamma, q_beta, k_gamma, k_beta, proj_w):
    x = np.asarray(x, np.float32)
    qkv_w = np.asarray(qkv_w, np.float32)
    proj_w = np.asarray(proj_w, np.float32)
    B = x.shape[0]
    import ml_dtypes
    xts = [np.ascontiguousarray(x[b].T).astype(ml_dtypes.bfloat16) for b in range(B)]
    gbs = []
    wqs = []
    pws = []
    for g in range(4):
        r = slice(192 * g, 192 * (g + 1))
        wq_rows = np.concatenate(
            [qkv_w[r], qkv_w[768:1536][r], qkv_w[1536:2304][r]], axis=0
        )
        # 6 extra columns: per-(q/k, head) row sums of W -> s1 = sum_d qkv
        wbar = np.stack(
            [wq_rows[64 * g : 64 * (g + 1)].sum(axis=0) for g in range(6)]
        )
        wq_rows = np.concatenate([wq_rows, wbar], axis=0)
        wqs.append(np.ascontiguousarray(wq_rows.T).astype(ml_dtypes.bfloat16))
        pws.append(np.ascontiguousarray(proj_w[:, r].T).astype(ml_dtypes.bfloat16))
        gbs.append(
            np.stack(
                [
                    np.asarray(q_gamma, np.float32),
                    np.asarray(q_beta, np.float32),
                    np.asarray(k_gamma, np.float32),
                    np.asarray(k_beta, np.float32),
                ]
            )
        )
    in_maps = []
    for core in range(8):
        b, g = core // 4, core % 4
        in_maps.append(
            {"x_t": xts[b], "wqkv_t": wqs[g], "projw_t": pws[g], "gb": gbs[g]}
        )
    return in_maps


def _is_plain_ln(q_gamma, q_beta, k_gamma, k_beta):
    return (
        np.all(np.asarray(q_gamma) == 1.0)
        and np.all(np.asarray(k_gamma) == 1.0)
        and np.all(np.asarray(q_beta) == 0.0)
        and np.all(np.asarray(k_beta) == 0.0)
    )


def run_cores(in_maps, N, trace=False, plain_ln=True):
    from concourse.bass_utils import run_bass_kernel_spmd

    nc = _built(N, plain_ln)
    res = run_bass_kernel_spmd(nc, in_maps, list(range(8)), trace=trace)
    return res


def kernel(x, qkv_w, q_gamma, q_beta, k_gamma, k_beta, proj_w, proj_b):
    x = np.asarray(x, np.float32)
    N = x.shape[1]
    plain = _is_plain_ln(q_gamma, q_beta, k_gamma, k_beta)
    in_maps = _prep_inputs(x, qkv_w, q_gamma, q_beta, k_gamma, k_beta, proj_w)
    res = run_cores(in_maps, N, plain_ln=plain)
    parts = [np.asarray(r["out_p"], np.float32) for r in res.results]
    out0 = parts[0] + parts[1] + parts[2] + parts[3]
    out1 = parts[4] + parts[5] + parts[6] + parts[7]
    out = np.stack([out0, out1]) + np.asarray(proj_b, np.float32)
    return out.astype(np.float32)
